# revision 1
# baseline (speedup 1.0000x reference)
"""Trainium2 Bass kernel for the DecoderSVM SNN decoder.

reference computation:
    curr[t,b,o] = einsum('bit,oi->tbo', inputs, W) + b         (I=182 -> O=2)
    syn_t = clip(alpha,0,1)*syn_{t-1} + curr_t                 (scan over T)
    mem_t = clip(beta,0,1)*mem_{t-1} + syn_t
    out = mem_rec transposed to [B, T, O]

Strategy (8 NeuronCores, batch-sharded 32 per core):
  - Block-diagonal GEMM: K=128 partitions = 32 batches x 4 input rows; the
    stationary lhsT [128, 64] holds W values block-diagonally so one matmul
    emits PSUM [64=(b,o), N] -- exactly the (batch,o)-per-partition layout
    the scan needs.  ceil(182/4) row-chunks accumulate the I contraction.
  - DMA: 4 chunks (16 input rows) per dma_start, with chunk c covering rows
    {base + 4i + c} so each SBUF partition receives one contiguous 4-row run
    from HBM (32KB f32 / 16KB bf16).
  - Bias enters PSUM via a rank-1 matmul: ones[1,N] x bias_row[1,64].
  - The double recurrence = two chained first-order linear scans done with
    VectorE's native tensor_tensor_scan (state = a*state + c) straight out
    of PSUM.
  - Output [64, 2000] DMAs contiguously; host reassembles [B, T, O].

Modes: "bf16" (default; host casts inputs, halves HBM traffic),
"bf16x3" (x/W split into bf16 hi+lo, 3 matmul passes, ~1e-5 rel err),
"f32r" (fp32 data, relaxed-precision matmul), "f32" (exact, PE-bound).
"""

import numpy as np

B, I, T, O = 256, 182, 2000, 2
NCORES = 8
NB = B // NCORES          # 32 batches per core
ROWS = 4                  # input rows folded into K per chunk
MERGE = 4                 # chunks per DMA (16 rows)
NGRP = 11                 # merged groups of MERGE chunks = 176 rows
EXTRA = 1                 # one extra plain 4-row chunk (rows 176..180)
NFULL = NGRP * MERGE + EXTRA   # 45 chunks of 4 rows
LAST_ROWS = I - NFULL * ROWS   # 2 rows in the K=64 tail chunk
M = 2 * NB                # 64 = output partitions (b_local, o)
TSPLIT = [512, 512, 512, 464]  # PSUM-bank-aligned time tiles

MODE = "bf16"
TRACE = False

_cache = {}


def _mode_cfg():
    """(np_dtype, matmul dtype name, n_passes)"""
    return {
        "f32": (np.float32, "float32", 1),
        "f32r": (np.float32, "float32r", 1),
        "bf16": ("bfloat16", "bfloat16", 1),
        "bf16x3": ("bfloat16", "bfloat16", 3),
    }[MODE]


def _np_dt():
    npdt, _, _ = _mode_cfg()
    if npdt == "bfloat16":
        import ml_dtypes

        return ml_dtypes.bfloat16
    return npdt


def chunk_rows(c):
    """Global input-row indices (length ROWS) covered by chunk c, matching the
    interleaved DMA layout: within a merged group, chunk cc covers rows
    base + 4*i + cc so partition (b, i) reads contiguous rows."""
    if c < NGRP * MERGE:
        g, cc = divmod(c, MERGE)
        base = g * ROWS * MERGE
        return [base + ROWS * i + cc for i in range(ROWS)]
    # plain trailing chunk(s): contiguous rows
    base = NGRP * MERGE * ROWS + (c - NGRP * MERGE) * ROWS
    return [base + i for i in range(ROWS)]


def _build_nc():
    import concourse.bacc as bacc
    import concourse.bass as bass
    import concourse.mybir as mybir
    from concourse.tile import TileContext

    f32 = mybir.dt.float32
    _, mdt_name, npasses = _mode_cfg()
    mdt = getattr(mybir.dt, mdt_name)
    # float32r memset is not encodable; the bias rank-1 matmul stays plain f32
    bdt = f32 if MODE == "f32r" else mdt

    nc = bacc.Bacc("TRN2", target_bir_lowering=False, debug=False)

    # x laid out [pass, NB, I, T]: pass 0 = hi, pass 1 = lo (bf16x3 only)
    nx = 2 if npasses > 1 else 1
    x = nc.dram_tensor("x", [nx, NB, I, T], mdt, kind="ExternalInput")
    lhsT_full = nc.dram_tensor(
        "lhsT_full", [128, npasses * NFULL * M], mdt, kind="ExternalInput"
    )
    lhsT_last = nc.dram_tensor(
        "lhsT_last", [2 * NB, npasses * M], mdt, kind="ExternalInput"
    )
    bias_row = nc.dram_tensor("bias_row", [1, M], bdt, kind="ExternalInput")
    alpha_bc = nc.dram_tensor("alpha_bc", [M, 512], f32, kind="ExternalInput")
    beta_bc = nc.dram_tensor("beta_bc", [M, 512], f32, kind="ExternalInput")
    y = nc.dram_tensor("y", [M, T], f32, kind="ExternalOutput")

    # which (pass, x-source) pairs each chunk runs: bf16x3 does
    # hi*W_hi + hi*W_lo + lo*W_hi
    passes = [(0, 0)] if npasses == 1 else [(0, 0), (1, 0), (2, 1)]

    with TileContext(nc) as tc:
        with (
            tc.tile_pool(name="consts", bufs=1) as cpool,
            tc.tile_pool(name="xs", bufs=4) as xpool,
            tc.tile_pool(name="xl", bufs=1) as xlpool,
            tc.tile_pool(name="mems", bufs=1) as mpool,
            tc.tile_pool(name="psum", bufs=1, space=bass.MemorySpace.PSUM) as ppool,
        ):
            lw = cpool.tile([128, npasses, NFULL, M], mdt)
            nc.sync.dma_start(out=lw[:], in_=lhsT_full[:])
            lwl = cpool.tile([2 * NB, npasses, M], mdt)
            nc.sync.dma_start(out=lwl[:], in_=lhsT_last[:])
            br = cpool.tile([1, M], bdt)
            nc.sync.dma_start(out=br[:], in_=bias_row[:])
            ab = cpool.tile([M, 512], f32)
            nc.sync.dma_start(out=ab[:], in_=alpha_bc[:])
            bb = cpool.tile([M, 512], f32)
            nc.sync.dma_start(out=bb[:], in_=beta_bc[:])
            ones = cpool.tile([1, T], bdt)
            nc.vector.memset(ones[:], 1.0)

            pt = ppool.tile([M, 2048], f32)

            first = True
            for xi in range(nx):
                dma_engines = [nc.sync, nc.scalar]
                for g in range(NGRP):
                    xt = xpool.tile([128, MERGE, T], mdt, tag="xt")
                    src = x[xi, :, g * ROWS * MERGE : (g + 1) * ROWS * MERGE, :]
                    src = src.rearrange(
                        "b (i cc) t -> b i cc t", i=ROWS, cc=MERGE
                    )
                    dma_engines[g % 2].dma_start(out=xt[:], in_=src)
                    for cc in range(MERGE):
                        c = g * MERGE + cc
                        for p, pxi in passes:
                            if pxi != xi:
                                continue
                            off = 0
                            for w in TSPLIT:
                                nc.tensor.matmul(
                                    pt[:, off : off + w],
                                    lw[:, p, c, :],
                                    xt[:, cc, off : off + w],
                                    start=first,
                                    stop=False,
                                )
                                off += w
                            first = False
                # trailing plain chunk (rows 176..180)
                c = NGRP * MERGE
                xe = xpool.tile([128, T], mdt, tag="xe")
                nc.sync.dma_start(
                    out=xe[:],
                    in_=x[xi, :, c * ROWS : c * ROWS + ROWS, :],
                )
                for p, pxi in passes:
                    if pxi != xi:
                        continue
                    off = 0
                    for w in TSPLIT:
                        nc.tensor.matmul(
                            pt[:, off : off + w],
                            lw[:, p, c, :],
                            xe[:, off : off + w],
                            start=False,
                            stop=False,
                        )
                        off += w
                # tail chunk: rows 180..182, K = 32 batches * 2 rows = 64
                xt2 = xlpool.tile([2 * NB, T], mdt, tag="xt2")
                nc.scalar.dma_start(out=xt2[:], in_=x[xi, :, NFULL * ROWS :, :])
                for p, pxi in passes:
                    if pxi != xi:
                        continue
                    off = 0
                    for w in TSPLIT:
                        nc.tensor.matmul(
                            pt[:, off : off + w],
                            lwl[:, p, :],
                            xt2[:, off : off + w],
                            start=False,
                            stop=False,
                        )
                        off += w
            # bias: ones[1, N] (x) bias_row[1, 64]
            off = 0
            for w in TSPLIT:
                nc.tensor.matmul(
                    pt[:, off : off + w],
                    br[:],
                    ones[:, off : off + w],
                    start=False,
                    stop=True,
                )
                off += w

            syn = mpool.tile([M, T], f32)
            mem = mpool.tile([M, T], f32)
            off = 0
            for ti, w in enumerate(TSPLIT):
                nc.vector.tensor_tensor_scan(
                    syn[:, off : off + w],
                    ab[:, :w],
                    pt[:, off : off + w],
                    initial=(0.0 if ti == 0 else syn[:, off - 1 : off]),
                    op0=mybir.AluOpType.mult,
                    op1=mybir.AluOpType.add,
                )
                off += w
            off = 0
            for ti, w in enumerate(TSPLIT):
                nc.vector.tensor_tensor_scan(
                    mem[:, off : off + w],
                    bb[:, :w],
                    syn[:, off : off + w],
                    initial=(0.0 if ti == 0 else mem[:, off - 1 : off]),
                    op0=mybir.AluOpType.mult,
                    op1=mybir.AluOpType.add,
                )
                off += w

            nc.sync.dma_start(out=y[:], in_=mem[:])

    nc.compile()
    return nc


def _split_hi_lo(a):
    """fp32 -> (hi, lo) bf16 pair with hi + lo ~= a."""
    import ml_dtypes

    hi = a.astype(ml_dtypes.bfloat16)
    lo = (a - hi.astype(np.float32)).astype(ml_dtypes.bfloat16)
    return hi, lo


def _host_tensors(W, b, alpha, beta):
    """Build the block-diagonal stationary weights + scan constant tensors."""
    npdt = _np_dt()
    _, _, npasses = _mode_cfg()
    W = np.asarray(W, np.float32)
    bvec = np.asarray(b, np.float32)
    a_cl = np.clip(np.asarray(alpha, np.float32), 0.0, 1.0)
    bt_cl = np.clip(np.asarray(beta, np.float32), 0.0, 1.0)

    if npasses > 1:
        W_hi, W_lo = _split_hi_lo(W)
        # pass p uses W variant: 0 -> hi, 1 -> lo, 2 -> hi
        W_per_pass = [
            W_hi.astype(np.float32),
            W_lo.astype(np.float32),
            W_hi.astype(np.float32),
        ]
    else:
        W_per_pass = [W]

    bidx = np.arange(NB)
    lhsT = np.zeros((128, npasses, NFULL, M), np.float32)
    lhsT_last = np.zeros((2 * NB, npasses, M), np.float32)
    for p in range(npasses):
        Wp = W_per_pass[p]
        for c in range(NFULL):
            rows = chunk_rows(c)
            for i in range(ROWS):
                for o in range(O):
                    lhsT[ROWS * bidx + i, p, c, 2 * bidx + o] = Wp[o, rows[i]]
        for i in range(LAST_ROWS):
            for o in range(O):
                lhsT_last[LAST_ROWS * bidx + i, p, 2 * bidx + o] = Wp[
                    o, NFULL * ROWS + i
                ]
    lhsT_full = lhsT.reshape(128, npasses * NFULL * M).astype(npdt)
    lhsT_last = lhsT_last.reshape(2 * NB, npasses * M).astype(npdt)

    bias_dt = np.float32 if MODE == "f32r" else npdt
    bias_row = np.tile(bvec, NB)[None, :].astype(bias_dt)
    alpha_bc = np.ascontiguousarray(
        np.broadcast_to(np.tile(a_cl, NB)[:, None], (M, 512))
    ).astype(np.float32)
    beta_bc = np.ascontiguousarray(
        np.broadcast_to(np.tile(bt_cl, NB)[:, None], (M, 512))
    ).astype(np.float32)
    return lhsT_full, lhsT_last, bias_row, alpha_bc, beta_bc


def kernel(inputs, W, b, alpha, beta):
    from concourse.bass_utils import run_bass_kernel_spmd

    key = MODE
    if key not in _cache:
        _cache[key] = _build_nc()
    nc = _cache[key]

    npdt = _np_dt()
    _, _, npasses = _mode_cfg()
    lhsT_full, lhsT_last, bias_row, alpha_bc, beta_bc = _host_tensors(
        W, b, alpha, beta
    )
    x_full = np.asarray(inputs, np.float32)
    if npasses > 1:
        x_hi, x_lo = _split_hi_lo(x_full)
        x_cast = np.stack([x_hi, x_lo])  # [2, B, I, T] bf16
    elif npdt != np.float32:
        x_cast = x_full.astype(npdt)[None]
    else:
        x_cast = x_full[None]

    in_maps = []
    for c in range(NCORES):
        in_maps.append(
            {
                "x": np.ascontiguousarray(x_cast[:, c * NB : (c + 1) * NB]),
                "lhsT_full": lhsT_full,
                "lhsT_last": lhsT_last,
                "bias_row": bias_row,
                "alpha_bc": alpha_bc,
                "beta_bc": beta_bc,
            }
        )

    res = run_bass_kernel_spmd(nc, in_maps, core_ids=list(range(NCORES)), trace=TRACE)
    kernel.last_exec_time_ns = res.exec_time_ns
    kernel.last_result = res
    out = np.empty((B, O, T), np.float32)
    for c in range(NCORES):
        out[c * NB : (c + 1) * NB] = res.results[c]["y"].reshape(NB, O, T)
    return np.ascontiguousarray(out.transpose(0, 2, 1))


kernel.last_exec_time_ns = None
kernel.last_result = None



# revision 3
# speedup vs baseline: 1.9397x; 1.9397x over previous
"""Trainium2 Bass kernel for the DecoderSVM SNN decoder (fp8 DoubleRow version).

reference computation:
    curr[t,b,o] = einsum('bit,oi->tbo', inputs, W) + b         (I=182 -> O=2)
    syn_t = clip(alpha,0,1)*syn_{t-1} + curr_t                 (scan over T)
    mem_t = clip(beta,0,1)*mem_{t-1} + syn_t
    out = mem_rec transposed to [B, T, O]

Strategy (8 NeuronCores, batch-sharded 32 per core), memory-bound so the
whole game is minimizing + streaming HBM bytes:

  - Inputs are shipped as fp8 e4m3 of (x - 0.5); the 0.5*sum(W)+b constant
    is folded into a rank-2 fp8 bias matmul (hi+lo split).  Host sim says
    rel_err ~3.7e-3 (vs 2e-2 gate).
  - Block-diagonal GEMM with perf_mode=DoubleRow: virtual K=256 = 32
    batches x 8 input rows (2 fp8 weights per PE cell), PSUM partitions
    m = 2*b_local + o.  23 weight units of [128, 2, 64] cover I=182 rows
    (2 zero-padded).
  - Time is split in 4 chunks [512, 512, 512, 464] (PSUM-bank sized).
    DMA, matmul, scan, and y-writeback pipeline chunk by chunk so the
    scans overlap the x stream of later chunks.
  - Host pre-arranges x into the exact SBUF layout ([128 partitions,
    46 cc-rows, C]) so every DMA is 128 x ~8KB contiguous descriptors.
  - The double recurrence = two chained tensor_tensor_scans per chunk.
"""

import numpy as np

B, I, T, O = 256, 182, 2000, 2
NCORES = 8
NB = B // NCORES  # 32 batches per core
M = 2 * NB  # 64 output partitions (b_local, o)
RR = 46  # cc-rows: 44 main (11 groups x 4) + 2 tail
NU = 23  # DoubleRow units (22 main + 1 tail)
TS = [512, 512, 512, 464]
OFFS = [0, 512, 1024, 1536]
SUBS = [(0, 16), (16, 32), (32, 46)]  # sub-DMA row splits

MODE = "fp8"
TRACE = False

_cache = {}


def _row_of(i, R):
    """Input row held by partition 4b+i at cc-row R (may be >=182 -> pad)."""
    if R < 44:
        g, cc = divmod(R, 4)
        return 16 * g + 4 * i + cc
    return 176 + 2 * i + (R - 44)


def _sub_of(u):
    """(sub_tile_index, local_row_offset) for unit u (covers cc-rows 2u, 2u+1)."""
    R = 2 * u
    for s, (r0, r1) in enumerate(SUBS):
        if R < r1:
            return s, R - r0
    raise AssertionError


def _build_nc():
    import concourse.bacc as bacc
    import concourse.bass as bass
    import concourse.mybir as mybir
    from concourse.tile import TileContext

    f32 = mybir.dt.float32
    f8 = mybir.dt.float8e4
    DR = mybir.MatmulPerfMode.DoubleRow

    nc = bacc.Bacc("TRN2", target_bir_lowering=False, debug=False)

    xd = [
        nc.dram_tensor(f"x{c}", [128, RR, C], f8, kind="ExternalInput")
        for c, C in enumerate(TS)
    ]
    lw = nc.dram_tensor("lw", [128, NU, 2, M], f8, kind="ExternalInput")
    bias2 = nc.dram_tensor("bias2", [2, M], f8, kind="ExternalInput")
    ones2 = nc.dram_tensor("ones2", [2, 512], f8, kind="ExternalInput")
    alpha_bc = nc.dram_tensor("alpha_bc", [M, 512], f32, kind="ExternalInput")
    beta_bc = nc.dram_tensor("beta_bc", [M, 512], f32, kind="ExternalInput")
    y = nc.dram_tensor("y", [M, T], f32, kind="ExternalOutput")

    with TileContext(nc) as tc:
        with (
            tc.tile_pool(name="consts", bufs=1) as cpool,
            tc.tile_pool(name="xs", bufs=12) as xpool,
            tc.tile_pool(name="mems", bufs=1) as mpool,
            tc.tile_pool(name="psum", bufs=4, space=bass.MemorySpace.PSUM) as ppool,
        ):
            # consts ride the gpsimd (SWDGE) queue so the two HWDGE queues
            # carry nothing but the x stream.
            b2 = cpool.tile([2, M], f8)
            nc.gpsimd.dma_start(out=b2[:], in_=bias2[:])
            on2 = cpool.tile([2, 512], f8)
            nc.gpsimd.dma_start(out=on2[:], in_=ones2[:])
            ab = cpool.tile([M, 512], f32)
            nc.gpsimd.dma_start(out=ab[:], in_=alpha_bc[:])
            bb = cpool.tile([M, 512], f32)
            nc.gpsimd.dma_start(out=bb[:], in_=beta_bc[:])
            lwt = cpool.tile([128, NU, 2, M], f8)
            nc.gpsimd.dma_start(out=lwt[:], in_=lw[:])

            syn = mpool.tile([M, T], f32)
            mem = mpool.tile([M, T], f32)

            # x stream: 12 sub-DMAs alternating over the two HWDGE queues
            qs = [nc.sync, nc.scalar]
            xt = {}
            qi = 0
            for c, C in enumerate(TS):
                for s, (r0, r1) in enumerate(SUBS):
                    t_ = xpool.tile([128, r1 - r0, C], f8, tag="xt", name=f"xt{c}{s}")
                    xt[(c, s)] = t_
                    qs[qi % 2].dma_start(out=t_[:], in_=xd[c][:, r0:r1, :])
                    qi += 1

            for c, C in enumerate(TS):
                off = OFFS[c]
                pt = ppool.tile([M, 512], f32, tag="pt", name=f"pt{c}")
                ptc = pt[:, :C]
                # bias first: starts the accumulation group, keeps PE warm
                nc.tensor.matmul(ptc, b2[:], on2[:, :C], start=True, stop=False)
                for u in range(NU):
                    s, lr = _sub_of(u)
                    rhs = xt[(c, s)][:, lr : lr + 2, :]
                    nc.tensor.matmul(
                        ptc,
                        lwt[:, u],
                        rhs,
                        start=False,
                        stop=(u == NU - 1),
                        perf_mode=DR,
                    )
                nc.vector.tensor_tensor_scan(
                    syn[:, off : off + C],
                    ab[:, :C],
                    ptc,
                    initial=(0.0 if c == 0 else syn[:, off - 1 : off]),
                    op0=mybir.AluOpType.mult,
                    op1=mybir.AluOpType.add,
                )
                nc.vector.tensor_tensor_scan(
                    mem[:, off : off + C],
                    bb[:, :C],
                    syn[:, off : off + C],
                    initial=(0.0 if c == 0 else mem[:, off - 1 : off]),
                    op0=mybir.AluOpType.mult,
                    op1=mybir.AluOpType.add,
                )
                nc.gpsimd.dma_start(out=y[:, off : off + C], in_=mem[:, off : off + C])

    nc.compile()
    return nc


def _host_tensors(W, b, alpha, beta):
    import ml_dtypes

    f8 = ml_dtypes.float8_e4m3
    W32 = np.asarray(W, np.float32)
    bvec = np.asarray(b, np.float32)
    Wq = W32.astype(f8).astype(np.float32)  # [O, I]

    lw = np.zeros((128, NU, 2, M), np.float32)
    for bb_ in range(NB):
        for i in range(4):
            p = 4 * bb_ + i
            for u in range(NU):
                for j in range(2):
                    r = _row_of(i, 2 * u + j)
                    if r < I:
                        for o in range(O):
                            lw[p, u, j, 2 * bb_ + o] = Wq[o, r]
    lw8 = lw.astype(f8)

    C = 0.5 * W32.sum(axis=1) + bvec  # [O] exact fold of the x-shift
    Chi = C.astype(f8).astype(np.float32)
    Clo = (C - Chi).astype(np.float32)
    bias2 = np.zeros((2, M), np.float32)
    bias2[0] = np.tile(Chi, NB)
    bias2[1] = np.tile(Clo, NB)
    bias2 = bias2.astype(f8)

    ones2 = np.ones((2, 512), np.float32).astype(f8)

    a_cl = np.clip(np.asarray(alpha, np.float32), 0.0, 1.0)
    bt_cl = np.clip(np.asarray(beta, np.float32), 0.0, 1.0)
    alpha_bc = np.ascontiguousarray(
        np.broadcast_to(np.tile(a_cl, NB)[:, None], (M, 512))
    ).astype(np.float32)
    beta_bc = np.ascontiguousarray(
        np.broadcast_to(np.tile(bt_cl, NB)[:, None], (M, 512))
    ).astype(np.float32)
    return lw8, bias2, ones2, alpha_bc, beta_bc


def _host_x(inputs):
    """Quantize (x-0.5) to e4m3 and pre-arrange into the per-core, per-chunk
    [128, 46, C] DMA layout."""
    import ml_dtypes

    f8 = ml_dtypes.float8_e4m3
    xs = np.asarray(inputs, np.float32) - 0.5
    xq = xs.astype(f8)  # [B, I, T]
    xqp = np.zeros((B, I + 2, T), f8)
    xqp[:, :I] = xq

    idx = np.empty((4, RR), np.int64)
    for i in range(4):
        for R in range(RR):
            idx[i, R] = _row_of(i, R)

    per_core = []
    for c in range(NCORES):
        arr = xqp[c * NB : (c + 1) * NB][:, idx, :]  # [NB, 4, RR, T]
        arr = arr.reshape(128, RR, T)
        chunks = {
            f"x{ci}": np.ascontiguousarray(arr[:, :, OFFS[ci] : OFFS[ci] + C])
            for ci, C in enumerate(TS)
        }
        per_core.append(chunks)
    return per_core


def kernel(inputs, W, b, alpha, beta):
    from concourse.bass_utils import run_bass_kernel_spmd

    key = "fp8"
    if key not in _cache:
        _cache[key] = _build_nc()
    nc = _cache[key]

    lw8, bias2, ones2, alpha_bc, beta_bc = _host_tensors(W, b, alpha, beta)
    per_core_x = _host_x(inputs)

    in_maps = []
    for c in range(NCORES):
        m = dict(per_core_x[c])
        m.update(
            lw=lw8, bias2=bias2, ones2=ones2, alpha_bc=alpha_bc, beta_bc=beta_bc
        )
        in_maps.append(m)

    res = run_bass_kernel_spmd(nc, in_maps, core_ids=list(range(NCORES)), trace=TRACE)
    kernel.last_exec_time_ns = res.exec_time_ns
    kernel.last_result = res
    out = np.empty((B, O, T), np.float32)
    for c in range(NCORES):
        out[c * NB : (c + 1) * NB] = res.results[c]["y"].reshape(NB, O, T)
    return np.ascontiguousarray(out.transpose(0, 2, 1))


kernel.last_exec_time_ns = None
kernel.last_result = None
